# revision 1
# baseline (speedup 1.0000x reference)
"""Causal multi-head attention on 8 Trainium2 NeuronCores.

Sharding: tensor-parallel across heads. 16 heads, 8 cores -> 2 heads/core.
Each core gets the full (pre-transposed) activations qT/kT/vT [C, B*T] and
its slice of the projection weights:
  WqT_c = Wq[c*128:(c+1)*128, :].T   [C, 128]   (128 = 2 heads * dk 64)
  WoT_c = Wo[:, c*128:(c+1)*128].T   [128, C]
Device computes the partial output  concat_c @ WoT_c  [B*T, C]; the host
sums the 8 partials (the "all-reduce after the output projection").

Device math per core (head dims on partitions, rows on the free axis):
  QHT_b = Wq_c @ qT[:, b]   [128, 2048]   (8 K-chunks of 128, N tiles of 512)
  KHT_b, VHT_b likewise; VHT is PE-transposed into VH [rows, dk] blocks with
  a ones-column appended so the PV matmul also produces softmax denominators.
  Per (batch b, local head l, 512-wide query group qg), kb = key block:
      ST[kb]  = KH @ QHT block            [128 keys, 512 queries]  (PSUM)
      P[kb]   = exp(ST * 1/8)             (ACT, PSUM->SBUF)
      P[kb]  *= tri_mask                  (diagonal blocks only)
      OT     += [VH|1].T @ P              [65, 512] PSUM accumulate
    (the PV accumulation lags the ST/exp chain by one kb so PE never stalls
     on ACT)
    OT[0:64] *= 1/OT[64]  (partition_broadcast of the reciprocal row)
  OUT rows = OTall_b.T @ WoT_c  streamed out per 512-row group; projection
  of batch 1 is emitted interleaved with attention of batch 0 so its DMA
  stream hides under attention compute.

float32r everywhere on the matmul path: full-rate PE (1 col/cycle at N=512)
at ~tf32 effective precision -- measured end-to-end rel err ~3e-4.
"""

import numpy as np

B, T, C = 2, 2048, 1024
H, DK = 16, 64
NCORES = 8
HL = H // NCORES          # local heads per core = 2
LD = HL * DK              # local head dims per core = 128
N = B * T                 # 4096 rows
KCH = C // 128            # 8 contraction chunks
QG = T // 512             # 4 query groups per batch
KB = T // 128             # 16 key blocks per batch

LAST_RESULTS = None       # BassKernelResults of the most recent run (for test.py)


def _build_program():
    import concourse.tile as tile
    import concourse.mybir as mybir
    from concourse import bacc
    from concourse.masks import make_identity
    from contextlib import ExitStack

    f32 = mybir.dt.float32
    f32r = mybir.dt.float32r
    EXP = mybir.ActivationFunctionType.Exp

    nc = bacc.Bacc("TRN2", target_bir_lowering=False, debug=False, num_devices=NCORES)
    qT_d = nc.declare_dram_parameter("qT", [C, N], f32r, isOutput=False)
    kT_d = nc.declare_dram_parameter("kT", [C, N], f32r, isOutput=False)
    vT_d = nc.declare_dram_parameter("vT", [C, N], f32r, isOutput=False)
    wq_d = nc.declare_dram_parameter("wqT", [C, LD], f32r, isOutput=False)
    wk_d = nc.declare_dram_parameter("wkT", [C, LD], f32r, isOutput=False)
    wv_d = nc.declare_dram_parameter("wvT", [C, LD], f32r, isOutput=False)
    wo_d = nc.declare_dram_parameter("woT", [LD, C], f32r, isOutput=False)
    mk_d = nc.declare_dram_parameter("masks", [128, 128], f32r, isOutput=False)
    out_d = nc.declare_dram_parameter("out", [N, C], f32, isOutput=True)

    with ExitStack() as ctx:
        tc = ctx.enter_context(tile.TileContext(nc))
        const = ctx.enter_context(tc.tile_pool(name="const", bufs=1))
        persist = ctx.enter_context(tc.tile_pool(name="persist", bufs=1))
        vhpool = ctx.enter_context(tc.tile_pool(name="vh", bufs=2))
        xpool = ctx.enter_context(tc.tile_pool(name="xt", bufs=12))
        ppool = ctx.enter_context(tc.tile_pool(name="p", bufs=8))
        opool = ctx.enter_context(tc.tile_pool(name="ot", bufs=4))
        spool = ctx.enter_context(tc.tile_pool(name="small", bufs=2))
        mmps = ctx.enter_context(tc.tile_pool(name="mmps", bufs=4, space="PSUM"))
        otps = ctx.enter_context(tc.tile_pool(name="otps", bufs=2, space="PSUM"))
        tps = ctx.enter_context(tc.tile_pool(name="tps", bufs=1, space="PSUM"))

        # ---- constants / weights ----
        wq = const.tile([128, KCH, LD], f32r)
        wk = const.tile([128, KCH, LD], f32r)
        wv = const.tile([128, KCH, LD], f32r)
        for w_t, w_dram in ((wq, wq_d), (wk, wk_d), (wv, wv_d)):
            for kk in range(KCH):
                nc.sync.dma_start(w_t[:, kk, :], w_dram[kk * 128:(kk + 1) * 128, :])
        wo = const.tile([128, C], f32r)
        nc.sync.dma_start(wo[:], wo_d[:])
        masks = const.tile([128, 128], f32r)
        nc.sync.dma_start(masks[:], mk_d[:])
        ident = const.tile([128, 128], f32)
        make_identity(nc, ident)

        # per-batch persistent activations (split so batch-1 projection DMA
        # can overlap batch-0 attention without WAR hazards)
        qht = [persist.tile([128, T], f32r, name=f"qht{b}") for b in range(B)]
        kht = [persist.tile([128, T], f32r, name=f"kht{b}") for b in range(B)]
        vht = [persist.tile([128, T], f32, name=f"vht{b}") for b in range(B)]
        otall = [persist.tile([128, T], f32r, name=f"otall{b}") for b in range(B)]

        def project_group(b, n):
            # one 512-row group of the k/v/q projections for batch b
            for w_t, src, dst in ((wk, kT_d, kht[b]), (wv, vT_d, vht[b]),
                                  (wq, qT_d, qht[b])):
                ps = mmps.tile([128, 512], f32, tag="proj", bufs=1)
                for kk in range(KCH):
                    xt = xpool.tile([128, 512], f32r)
                    nc.sync.dma_start(
                        xt[:], src[kk * 128:(kk + 1) * 128,
                                   b * T + n * 512: b * T + (n + 1) * 512])
                    nc.tensor.matmul(ps[:], w_t[:, kk, :], xt[:],
                                     start=(kk == 0), stop=(kk == KCH - 1))
                nc.vector.tensor_copy(dst[:, n * 512:(n + 1) * 512], ps[:])

        def alloc_vh(b):
            # VH blocks [128 rows, 64] per local head, ones col appended
            vh = [vhpool.tile([128, KB, 65], f32r, tag=f"vh{l}", name=f"vh{l}_{b}")
                  for l in range(HL)]
            for l in range(HL):
                nc.vector.memset(vh[l][:, :, 64:65].bitcast(f32), 1.0)
            return vh

        def make_vh_group(b, vh, n):
            # transpose VHT key blocks 4n..4n+3 into the VH tiles
            for kb in range(4 * n, 4 * n + 4):
                tp = tps.tile([128, 128], f32)
                nc.tensor.transpose(
                    tp[:], vht[b][:, kb * 128:(kb + 1) * 128], ident[:])
                for l in range(HL):
                    nc.vector.tensor_copy(vh[l][:, kb, 0:64], tp[:, l * 64:(l + 1) * 64])

        def attention_qg(b, vh, qg):
            q0 = qg * 512
            nkb = 4 * qg + 4
            # both local heads' ST/exp/PV chains interleaved at strip level:
            # PE sees 4 matmuls per ACT window instead of 2, and the two
            # chains hide each other's exp latency. PV lags ST/exp by one kb.
            otp = [otps.tile([65, 512], f32, tag="otp", name=f"otp_{b}_{qg}_{l}")
                   for l in range(HL)]
            ps = [[], []]
            for kb in range(nkb):
                for l in range(HL):
                    hs = slice(l * 64, (l + 1) * 64)
                    st = mmps.tile([128, 512], f32, tag="mm",
                                   name=f"st_{b}_{qg}_{l}_{kb}")
                    nc.tensor.matmul(
                        st[:],
                        kht[b][hs, kb * 128:(kb + 1) * 128],
                        qht[b][hs, q0: q0 + 512],
                        start=True, stop=True)
                    p = ppool.tile([128, 512], f32r, tag="p",
                                   name=f"p_{b}_{qg}_{l}_{kb}")
                    if kb >= 4 * qg:
                        # diagonal strip: cols < 128*d are fully masked, the
                        # [128d, 128d+128) block is the causal triangle
                        d = kb - 4 * qg
                        if d > 0:
                            nc.vector.memset(p[:, 0:128 * d].bitcast(f32), 0.0)
                        nc.scalar.activation(p[:, 128 * d:512], st[:, 128 * d:512],
                                             EXP, scale=0.125)
                        nc.vector.tensor_mul(
                            p[:, 128 * d:128 * d + 128],
                            p[:, 128 * d:128 * d + 128], masks[:])
                    else:
                        nc.scalar.activation(p[:], st[:], EXP, scale=0.125)
                    ps[l].append(p)
                if kb >= 2:
                    for l in range(HL):
                        nc.tensor.matmul(otp[l][:], vh[l][:, kb - 2, :],
                                         ps[l][kb - 2][:],
                                         start=(kb == 2), stop=False)
            for kb in (nkb - 2, nkb - 1):
                for l in range(HL):
                    nc.tensor.matmul(otp[l][:], vh[l][:, kb, :], ps[l][kb][:],
                                     start=(kb == 0), stop=(kb == nkb - 1))
            for l in range(HL):
                hs = slice(l * 64, (l + 1) * 64)
                recip = spool.tile([1, 512], f32, tag="recip")
                nc.vector.reciprocal(recip[:], otp[l][64:65, :])
                rep = spool.tile([64, 512], f32, tag="rep")
                nc.gpsimd.partition_broadcast(rep[:], recip[:])
                with nc.allow_low_precision(reason="round for f32r out-proj"):
                    nc.vector.tensor_mul(otall[b][hs, q0: q0 + 512],
                                         otp[l][0:64, :], rep[:])
        def outproj_qg(b, qg):
            # output projection + store for this 512-row group
            q0 = qg * 512
            for rt in range(4):
                row0 = q0 + rt * 128
                for nn in range(2):
                    ops = mmps.tile([128, 512], f32, tag="mm")
                    nc.tensor.matmul(ops[:], otall[b][:, row0:row0 + 128],
                                     wo[:, nn * 512:(nn + 1) * 512],
                                     start=True, stop=True)
                    ot = opool.tile([128, 512], f32, tag="ot")
                    if nn == 0:
                        nc.vector.tensor_copy(ot[:], ops[:])
                    else:
                        nc.scalar.copy(ot[:], ops[:])
                    nc.sync.dma_start(
                        out_d[b * T + row0: b * T + row0 + 128,
                              nn * 512:(nn + 1) * 512], ot[:])

        # group-granular interleave: DMA streams both batches continuously;
        # attention of either batch starts as soon as its key/query groups land
        vh0, vh1 = alloc_vh(0), alloc_vh(1)
        for n in range(QG):
            project_group(0, n)
            make_vh_group(0, vh0, n)
            project_group(1, n)
            make_vh_group(1, vh1, n)
            attention_qg(0, vh0, n)
            attention_qg(1, vh1, n)
            outproj_qg(0, n)
            outproj_qg(1, n)

    nc.compile()
    return nc


def _make_masks():
    j = np.arange(128)[None, :]
    p = np.arange(128)[:, None]
    return (j >= p).astype(np.float32)


def kernel(q, k, v, Wq, Wk, Wv, Wo):
    global LAST_RESULTS
    from concourse.bass_utils import run_bass_kernel_spmd

    q = np.ascontiguousarray(np.asarray(q, np.float32).reshape(N, C).T)
    k = np.ascontiguousarray(np.asarray(k, np.float32).reshape(N, C).T)
    v = np.ascontiguousarray(np.asarray(v, np.float32).reshape(N, C).T)
    Wq = np.asarray(Wq, np.float32)
    Wk = np.asarray(Wk, np.float32)
    Wv = np.asarray(Wv, np.float32)
    Wo = np.asarray(Wo, np.float32)
    masks = _make_masks()

    in_maps = []
    for c in range(NCORES):
        sl = slice(c * LD, (c + 1) * LD)
        in_maps.append({
            "qT": q, "kT": k, "vT": v,
            "wqT": np.ascontiguousarray(Wq[sl, :].T),
            "wkT": np.ascontiguousarray(Wk[sl, :].T),
            "wvT": np.ascontiguousarray(Wv[sl, :].T),
            "woT": np.ascontiguousarray(Wo[:, sl].T),
            "masks": masks,
        })

    nc = _build_program()
    res = run_bass_kernel_spmd(nc, in_maps, list(range(NCORES)))
    LAST_RESULTS = res
    acc = np.zeros((N, C), np.float64)
    for rmap in res.results:
        acc += rmap["out"]
    return acc.astype(np.float32).reshape(B, T, C)



# revision 3
# speedup vs baseline: 1.1712x; 1.1712x over previous
"""Causal multi-head attention on 8 Trainium2 NeuronCores.

Sharding: tensor-parallel across heads. 16 heads, 8 cores -> 2 heads/core.
Each core gets the full (pre-transposed, bf16) activations qT/kT/vT and its
slice of the projection weights; it computes the partial output
concat_c @ WoT_c [B*T, C]; the host sums the 8 partials (the "all-reduce
after the output projection").

Numerics: bf16 operands on the whole matmul path, fp32 PSUM accumulation,
softmax in fp32 (exp on ACT, denominators via a ones-column folded into the
PV matmul).  Measured end-to-end rel err ~5e-3 vs the fp32 reference.

Device math per core (head dims on partitions, rows on the free axis):
  QHT_b = Wq_c @ qT[:, b]  [128, 2048] bf16; KHT, VHT likewise.  VHT is
  PE-transposed (bf16) into VH [keys, dk] blocks with a ones column so the
  PV matmul also produces softmax denominators.
  Per (batch b, 512-wide query group qg), kb = key block, both heads l:
      ST[l]  = KH_l @ QHT_l block   [128 keys, 512-c0 queries] (paired PSUM
               banks; c0 = causal left-trim for diagonal blocks)
      P      = exp(ST * 1/8)        one ACT per kb covering both heads
      causal mask applied by gpsimd.affine_select (per-partition threshold)
      OT[l] += [VH_l|1].T @ P[l]    [65, 512-c0] PSUM accumulate
    (PV lags the ST/exp chain by two kb so PE never stalls on ACT)
    OT[0:64] *= 1/OT[64]  (reciprocal_approx_fast + partition_broadcast)
  OUT rows = OTall_b.T @ WoT_c, streamed out bf16 per 128-row group.
"""

import numpy as np

B, T, C = 2, 2048, 1024
H, DK = 16, 64
NCORES = 8
HL = H // NCORES          # local heads per core = 2
LD = HL * DK              # local head dims per core = 128
N = B * T                 # 4096 rows
KCH = C // 128            # 8 contraction chunks
QG = T // 512             # 4 query groups per batch
KB = T // 128             # 16 key blocks per batch

LAST_RESULTS = None       # BassKernelResults of the most recent run (for test.py)


def _build_program():
    import concourse.tile as tile
    import concourse.mybir as mybir
    from concourse import bacc
    from concourse.masks import make_identity
    from contextlib import ExitStack

    f32 = mybir.dt.float32
    bf16 = mybir.dt.bfloat16
    EXP = mybir.ActivationFunctionType.Exp

    nc = bacc.Bacc("TRN2", target_bir_lowering=False, debug=False, num_devices=NCORES)
    # activations pre-chunked on host: x[p, kk, t] = xT[kk*128+p, t]
    qT_d = nc.declare_dram_parameter("qT", [128, KCH, N], bf16, isOutput=False)
    kT_d = nc.declare_dram_parameter("kT", [128, KCH, N], bf16, isOutput=False)
    vT_d = nc.declare_dram_parameter("vT", [128, KCH, N], bf16, isOutput=False)
    # weights pre-chunked on host: w[p, kk, l] = WT[kk*128+p, l]
    wq_d = nc.declare_dram_parameter("wqT", [128, KCH, LD], bf16, isOutput=False)
    wk_d = nc.declare_dram_parameter("wkT", [128, KCH, LD], bf16, isOutput=False)
    wv_d = nc.declare_dram_parameter("wvT", [128, KCH, LD], bf16, isOutput=False)
    wo_d = nc.declare_dram_parameter("woT", [LD, C], bf16, isOutput=False)
    out_d = nc.declare_dram_parameter("out", [N, C], bf16, isOutput=True)

    with ExitStack() as ctx:
        tc = ctx.enter_context(tile.TileContext(nc))
        const = ctx.enter_context(tc.tile_pool(name="const", bufs=1))
        persist = ctx.enter_context(tc.tile_pool(name="persist", bufs=1))
        xpool = ctx.enter_context(tc.tile_pool(name="xt", bufs=4))
        ppool = ctx.enter_context(tc.tile_pool(name="p", bufs=6))
        opool = ctx.enter_context(tc.tile_pool(name="ot", bufs=4))
        spool = ctx.enter_context(tc.tile_pool(name="small", bufs=2))
        stps = ctx.enter_context(tc.tile_pool(name="stps", bufs=2, space="PSUM"))
        otps = ctx.enter_context(tc.tile_pool(name="otps", bufs=2, space="PSUM"))
        mps = ctx.enter_context(tc.tile_pool(name="mps", bufs=2, space="PSUM"))

        # ---- constants / weights ----
        wq = const.tile([128, KCH, LD], bf16)
        wk = const.tile([128, KCH, LD], bf16)
        wv = const.tile([128, KCH, LD], bf16)
        for w_t, w_dram in ((wq, wq_d), (wk, wk_d), (wv, wv_d)):
            nc.sync.dma_start(w_t[:], w_dram[:])
        wo = const.tile([128, C], bf16)
        nc.sync.dma_start(wo[:], wo_d[:])
        ident = const.tile([128, 128], bf16)
        make_identity(nc, ident)

        # per-batch persistent activations
        qht = [persist.tile([128, T], bf16, name=f"qht{b}") for b in range(B)]
        kht = [persist.tile([128, T], bf16, name=f"kht{b}") for b in range(B)]
        vht = [persist.tile([128, T], bf16, name=f"vht{b}") for b in range(B)]
        otall = [persist.tile([128, T], bf16, name=f"otall{b}") for b in range(B)]
        # VH blocks [keys, dk] per (key block, local head), ones col appended
        vh = [persist.tile([128, KB, HL, 65], bf16, name=f"vh{b}") for b in range(B)]
        for b in range(B):
            nc.vector.memset(vh[b][:, :, :, 64:65], 1.0)

        def project_group(b, n):
            # one 512-row group of the k/v/q projections for batch b
            cols = slice(b * T + n * 512, b * T + (n + 1) * 512)
            for w_t, src, dst in ((wk, kT_d, kht[b]), (wv, vT_d, vht[b]),
                                  (wq, qT_d, qht[b])):
                xt = xpool.tile([128, KCH, 512], bf16)
                nc.sync.dma_start(xt[:], src[:, :, cols])
                ps = mps.tile([128, 512], f32, tag="mm")
                for kk in range(KCH):
                    nc.tensor.matmul(ps[:], w_t[:, kk, :], xt[:, kk, :],
                                     start=(kk == 0), stop=(kk == KCH - 1))
                nc.vector.tensor_copy(dst[:, n * 512:(n + 1) * 512], ps[:])

        def make_vh_group(b, n):
            # transpose VHT key blocks 4n..4n+3 into the VH tiles (both heads
            # of one transpose land via a single strided copy)
            for kb in range(4 * n, 4 * n + 4):
                tp = mps.tile([128, 128], bf16, tag="mm")
                nc.tensor.transpose(
                    tp[:], vht[b][:, kb * 128:(kb + 1) * 128], ident[:])
                nc.vector.tensor_copy(vh[b][:, kb, :, 0:64], tp[:])

        def attention_qg(b, qg):
            q0 = qg * 512
            nkb = 4 * qg + 4
            otp = [otps.tile([65, 512], f32, tag="otp", name=f"otp_{b}_{qg}_{l}")
                   for l in range(HL)]
            ps = []

            def pv(kb):
                p, c0 = ps[kb]
                for l in range(HL):
                    nc.tensor.matmul(otp[l][:, c0:512], vh[b][:, kb, l, :],
                                     p[:, l, c0:512],
                                     start=(kb == 0), stop=(kb == nkb - 1))

            for kb in range(nkb):
                d = kb - 4 * qg
                c0 = 128 * d if d > 0 else 0
                st = stps.tile([128, HL, 512], f32, tag="st",
                               name=f"st_{b}_{qg}_{kb}")
                for l in range(HL):
                    hs = slice(l * 64, (l + 1) * 64)
                    nc.tensor.matmul(
                        st[:, l, c0:512],
                        kht[b][hs, kb * 128:(kb + 1) * 128],
                        qht[b][hs, q0 + c0: q0 + 512],
                        start=True, stop=True)
                p = ppool.tile([128, HL, 512], bf16, tag="p",
                               name=f"p_{b}_{qg}_{kb}")
                nc.scalar.activation(p[:, :, c0:512], st[:, :, c0:512],
                                     EXP, scale=0.125)
                if d >= 0:
                    # zero keys below the causal diagonal: keep where
                    # query_col >= key_partition + 128*d
                    nc.gpsimd.affine_select(
                        out=p[:], in_=p[:],
                        compare_op=mybir.AluOpType.is_ge,
                        fill=0.0, base=-128 * d, channel_multiplier=-1,
                        pattern=[[0, HL], [1, 512]])
                ps.append((p, c0))
                if kb >= 2:
                    pv(kb - 2)
            pv(nkb - 2)
            pv(nkb - 1)

            for l in range(HL):
                recip = spool.tile([1, 512], f32, tag="recip")
                nc.vector.reciprocal_approx_fast(recip[:], otp[l][64:65, :])
                rep = spool.tile([64, 512], f32, tag="rep")
                nc.gpsimd.partition_broadcast(rep[:], recip[:])
                with nc.allow_low_precision(reason="bf16 out of f32 softmax"):
                    nc.vector.tensor_mul(
                        otall[b][l * 64:(l + 1) * 64, q0: q0 + 512],
                        otp[l][0:64, :], rep[:])

        def outproj_qg(b, qg):
            # output projection + store for this 512-row group
            q0 = qg * 512
            for rt in range(4):
                row0 = q0 + rt * 128
                ot = opool.tile([128, 2, 512], bf16, tag="ot")
                for nn in range(2):
                    ops = mps.tile([128, 512], f32, tag="mm")
                    nc.tensor.matmul(ops[:], otall[b][:, row0:row0 + 128],
                                     wo[:, nn * 512:(nn + 1) * 512],
                                     start=True, stop=True)
                    nc.vector.tensor_copy(ot[:, nn, :], ops[:])
                nc.sync.dma_start(
                    out_d[b * T + row0: b * T + row0 + 128, :], ot[:])

        # group-granular interleave: DMA streams both batches continuously;
        # attention of either batch starts as soon as its groups land
        for n in range(QG):
            for b in range(B):
                project_group(b, n)
                make_vh_group(b, n)
            for b in range(B):
                attention_qg(b, n)
            for b in range(B):
                outproj_qg(b, n)

    nc.compile()
    return nc


def kernel(q, k, v, Wq, Wk, Wv, Wo):
    global LAST_RESULTS
    import ml_dtypes
    from concourse.bass_utils import run_bass_kernel_spmd

    bf16 = ml_dtypes.bfloat16

    def chunk_T(x):
        # [N, C] -> xT [C, N] -> [128, KCH, N] with x[p, kk, t] = xT[kk*128+p, t]
        xT = np.asarray(x, np.float32).reshape(N, C).T
        return np.ascontiguousarray(
            xT.reshape(KCH, 128, N).transpose(1, 0, 2)).astype(bf16)

    qc, kc, vc = chunk_T(q), chunk_T(k), chunk_T(v)
    Wq = np.asarray(Wq, np.float32)
    Wk = np.asarray(Wk, np.float32)
    Wv = np.asarray(Wv, np.float32)
    Wo = np.asarray(Wo, np.float32)

    def chunk_W(W, sl):
        # Wc = W[sl, :].T [C, LD] -> [128, KCH, LD]
        WT = W[sl, :].T
        return np.ascontiguousarray(
            WT.reshape(KCH, 128, LD).transpose(1, 0, 2)).astype(bf16)

    in_maps = []
    for c in range(NCORES):
        sl = slice(c * LD, (c + 1) * LD)
        in_maps.append({
            "qT": qc, "kT": kc, "vT": vc,
            "wqT": chunk_W(Wq, sl),
            "wkT": chunk_W(Wk, sl),
            "wvT": chunk_W(Wv, sl),
            "woT": np.ascontiguousarray(Wo[:, sl].T).astype(bf16),
        })

    nc = _build_program()
    res = run_bass_kernel_spmd(nc, in_maps, list(range(NCORES)))
    LAST_RESULTS = res
    acc = np.zeros((N, C), np.float32)
    for rmap in res.results:
        acc += np.asarray(rmap["out"], np.float32)
    return acc.reshape(B, T, C)


# revision 31
# speedup vs baseline: 1.5815x; 1.3504x over previous
"""Causal multi-head attention on 8 Trainium2 NeuronCores.

Sharding: tensor-parallel across heads. 16 heads, 8 cores -> 2 heads/core.
Each core gets the full (pre-transposed, bf16) activations qT/kT/vT and its
slice of the projection weights; it computes the partial output
concat_c @ WoT_c [B*T, C]; the host sums the 8 partials (the "all-reduce
after the output projection").

Numerics: bf16 operands on the whole matmul path, fp32 PSUM accumulation,
softmax in fp32 (exp on ACT, denominators via a ones-column folded into the
PV matmul).  Measured end-to-end rel err ~5e-3 vs the fp32 reference.

Device math per core (head dims on partitions, rows on the free axis):
  QHT_b = Wq_c @ qT[:, b]  [128, 2048] bf16; KHT, VHT likewise.  VHT is
  PE-transposed (bf16) into VH [keys, dk] blocks with a ones column so the
  PV matmul also produces softmax denominators.
  Per (batch b, 512-wide query group qg), kb = key block, both heads l:
      ST[l]  = KH_l @ QHT_l block   [128 keys, 512-c0 queries] (paired PSUM
               banks; c0 = causal left-trim for diagonal blocks)
      P      = exp(ST * 1/8)        one ACT per kb covering both heads
      causal mask applied by gpsimd.affine_select (per-partition threshold)
      OT[l] += [VH_l|1].T @ P[l]    [65, 512-c0] PSUM accumulate
    (PV lags the ST/exp chain by two kb so PE never stalls on ACT)
    OT[0:64] *= 1/OT[64]  (reciprocal_approx_fast + partition_broadcast)
  OUT rows = OTall_b.T @ WoT_c, streamed out bf16 per 128-row group.
"""

import numpy as np

B, T, C = 2, 2048, 1024
H, DK = 16, 64
NCORES = 8
HL = H // NCORES          # local heads per core = 2
LD = HL * DK              # local head dims per core = 128
N = B * T                 # 4096 rows
KCH = C // 128            # 8 contraction chunks
QG = T // 512             # 4 query groups per batch
KB = T // 128             # 16 key blocks per batch

LAST_RESULTS = None       # BassKernelResults of the most recent run (for test.py)


def _build_program():
    import concourse.tile as tile
    import concourse.mybir as mybir
    from concourse import bacc
    from contextlib import ExitStack

    f32 = mybir.dt.float32
    bf16 = mybir.dt.bfloat16
    EXP = mybir.ActivationFunctionType.Exp

    nc = bacc.Bacc("TRN2", target_bir_lowering=False, debug=False, num_devices=NCORES)
    # activations pre-chunked on host: x[p, kk, t] = xT[kk*128+p, t]
    qT_d = nc.declare_dram_parameter("qT", [128, KCH, N], bf16, isOutput=False)
    kT_d = nc.declare_dram_parameter("kT", [128, KCH, N], bf16, isOutput=False)
    vT_d = nc.declare_dram_parameter("vT", [128, KCH, N], bf16, isOutput=False)
    # weights pre-chunked on host: w[p, kk, l] = WT[kk*128+p, l]
    wq_d = nc.declare_dram_parameter("wqT", [128, KCH, LD], bf16, isOutput=False)
    wk_d = nc.declare_dram_parameter("wkT", [128, KCH, LD], bf16, isOutput=False)
    wv_d = nc.declare_dram_parameter("wvT", [128, KCH, LD], bf16, isOutput=False)
    wo_d = nc.declare_dram_parameter("woT", [LD, C], bf16, isOutput=False)
    out_d = nc.declare_dram_parameter("out", [N, C], bf16, isOutput=True)

    with ExitStack() as ctx:
        tc = ctx.enter_context(tile.TileContext(nc))
        const = ctx.enter_context(tc.tile_pool(name="const", bufs=1))
        persist = ctx.enter_context(tc.tile_pool(name="persist", bufs=1))
        xpool = ctx.enter_context(tc.tile_pool(name="xt", bufs=4))
        ppool = ctx.enter_context(tc.tile_pool(name="p", bufs=8))
        opool = ctx.enter_context(tc.tile_pool(name="ot", bufs=4))
        spool = ctx.enter_context(tc.tile_pool(name="small", bufs=2))
        stps = ctx.enter_context(tc.tile_pool(name="stps", bufs=2, space="PSUM"))
        otps = ctx.enter_context(tc.tile_pool(name="otps", bufs=2, space="PSUM"))
        mps = ctx.enter_context(tc.tile_pool(name="mps", bufs=2, space="PSUM"))

        # ---- constants / weights: each weight's DMA is emitted just before
        # its first consumer so the first k-projection chunk doesn't queue
        # behind unrelated weight transfers on the DMA engines ----
        wq = const.tile([128, KCH, LD], bf16)
        wk = const.tile([128, KCH, LD], bf16)
        wv = const.tile([128, KCH, LD], bf16)
        wo = const.tile([128, C], bf16)
        w_dram_of = {id(wk): wk_d, id(wv): wv_d, id(wq): wq_d, id(wo): wo_d}
        w_loaded = set()

        def load_weight(w_t):
            if id(w_t) not in w_loaded:
                w_loaded.add(id(w_t))
                nc.sync.dma_start(w_t[:], w_dram_of[id(w_t)][:])

        # per-batch persistent activations
        qht = [persist.tile([128, T], bf16, name=f"qht{b}") for b in range(B)]
        kht = [persist.tile([128, T], bf16, name=f"kht{b}") for b in range(B)]
        otall = [persist.tile([128, T], bf16, name=f"otall{b}") for b in range(B)]
        # VH blocks [keys, dk] per (key block, local head), ones col appended
        vh = [persist.tile([128, KB, HL, 65], bf16, name=f"vh{b}") for b in range(B)]
        for b in range(B):
            nc.vector.memset(vh[b][:, :, :, 64:65], 1.0)

        loads = {}

        def load_group(b, n):
            # DMA issue only (SP queue): k/q/v input chunks for group (b, n),
            # each as two half-DMAs so the matmul chains can start after the
            # first half
            cols = slice(b * T + n * 512, b * T + (n + 1) * 512)
            tiles = {}
            for key, w_t, src in (("k", wk, kT_d), ("q", wq, qT_d),
                                  ("v", wv, vT_d)):
                load_weight(w_t)
                xh = [xpool.tile([128, KCH // 2, 512], bf16,
                                 name=f"xh_{key}{i}", tag=f"xh_{key}{i}")
                      for i in range(2)]
                for i in range(2):
                    nc.sync.dma_start(
                        xh[i][:], src[:, i * (KCH // 2):(i + 1) * (KCH // 2), cols])
                tiles[key] = xh
            loads[(b, n)] = tiles

        def project_group(b, n):
            # one 512-row group of the k/q projections (head dims on
            # partitions) and the v projection directly in key-major
            # orientation (tokens on partitions — no PE transpose needed)
            tiles = loads.pop((b, n))
            for key, w_t, dst in (("k", wk, kht[b]), ("q", wq, qht[b])):
                xh = tiles[key]
                ps = mps.tile([128, 512], f32, tag="mm")
                for kk in range(KCH):
                    nc.tensor.matmul(ps[:], w_t[:, kk, :],
                                     xh[kk // (KCH // 2)][:, kk % (KCH // 2), :],
                                     start=(kk == 0), stop=(kk == KCH - 1))
                nc.vector.tensor_copy(dst[:, n * 512:(n + 1) * 512], ps[:])
            # v: VH[tok, ld] = sum_c vT[c, tok] * WvT[c, ld], per key block
            vxh = tiles["v"]
            for j in range(4):
                kb = 4 * n + j
                ps = mps.tile([128, 128], f32, tag="mm")
                for kk in range(KCH):
                    nc.tensor.matmul(
                        ps[:],
                        vxh[kk // (KCH // 2)][:, kk % (KCH // 2),
                                              j * 128:(j + 1) * 128],
                        wv[:, kk, :],
                        start=(kk == 0), stop=(kk == KCH - 1))
                nc.vector.tensor_copy(vh[b][:, kb, :, 0:64], ps[:])

        def attention_qg(b, qg):
            q0 = qg * 512
            nkb = 4 * qg + 4
            otp = [otps.tile([65, 512], f32, tag="otp", name=f"otp_{b}_{qg}_{l}")
                   for l in range(HL)]
            ps = []

            def pv(kb):
                p, c0 = ps[kb]
                for l in range(HL):
                    nc.tensor.matmul(otp[l][:, c0:512], vh[b][:, kb, l, :],
                                     p[:, l, c0:512],
                                     start=(kb == 0), stop=(kb == nkb - 1))

            for kb in range(nkb):
                d = kb - 4 * qg
                c0 = 128 * d if d > 0 else 0
                st = stps.tile([128, HL, 512], f32, tag="st",
                               name=f"st_{b}_{qg}_{kb}")
                for l in range(HL):
                    hs = slice(l * 64, (l + 1) * 64)
                    nc.tensor.matmul(
                        st[:, l, c0:512],
                        kht[b][hs, kb * 128:(kb + 1) * 128],
                        qht[b][hs, q0 + c0: q0 + 512],
                        start=True, stop=True)
                p = ppool.tile([128, HL, 512], bf16, tag="p",
                               name=f"p_{b}_{qg}_{kb}")
                nc.scalar.activation(p[:, :, c0:512], st[:, :, c0:512],
                                     EXP, scale=0.125)
                if d >= 0:
                    # zero keys below the causal diagonal within the live
                    # region [c0:512]: with j = col - c0 and c0 = 128*d the
                    # condition key<=query becomes j >= key_partition
                    nc.gpsimd.affine_select(
                        out=p[:, :, c0:512], in_=p[:, :, c0:512],
                        compare_op=mybir.AluOpType.is_ge,
                        fill=0.0, base=0, channel_multiplier=-1,
                        pattern=[[0, HL], [1, 512 - c0]])
                ps.append((p, c0))
                if kb >= 3:
                    pv(kb - 3)
            pv(nkb - 3)
            pv(nkb - 2)
            pv(nkb - 1)

            def rescale():
                # deferred to the next iteration: by then the otp data is
                # long-ready, so these never head-of-line-block the DVE queue
                for l in range(HL):
                    recip = spool.tile([1, 512], f32, tag="recip")
                    nc.vector.reciprocal(recip[:], otp[l][64:65, :])
                    rep = spool.tile([64, 512], f32, tag="rep")
                    nc.gpsimd.partition_broadcast(rep[:], recip[:])
                    with nc.allow_low_precision(reason="bf16 out of f32 softmax"):
                        nc.vector.tensor_mul(
                            otall[b][l * 64:(l + 1) * 64, q0: q0 + 512],
                            otp[l][0:64, :], rep[:])
            return rescale

        def outproj_qg(b, qg, tail=False):
            # output projection + store for this 512-row group; evacuation
            # copies alternate DVE/ACT so neither queue head-of-line blocks
            # (in the drain tail DVE is stuck behind the final softmax
            # rescale, so everything goes to ACT there); each half is DMA'd
            # as soon as its copy lands
            load_weight(wo)
            q0 = qg * 512
            # the out-DMA is issued from the same queue as the evacuation
            # copy: it needs no extra semaphore wait there, and keeps the SP
            # queue free for input prefetch (an out-DMA waiting on its copy
            # otherwise blocks all later input DMAs behind it)
            use_act = tail and b == 0
            for rt in range(4):
                row0 = q0 + rt * 128
                ot = opool.tile([128, 2, 512], bf16, tag="ot")
                for nn in range(2):
                    ops = mps.tile([128, 512], f32, tag="mm")
                    nc.tensor.matmul(ops[:], otall[b][:, row0:row0 + 128],
                                     wo[:, nn * 512:(nn + 1) * 512],
                                     start=True, stop=True)
                    if use_act:
                        nc.scalar.copy(ot[:, nn, :], ops[:])
                    else:
                        nc.vector.tensor_copy(ot[:, nn, :], ops[:])
                eng = nc.scalar if use_act else nc.sync
                eng.dma_start(
                    out_d[b * T + row0: b * T + row0 + 128, :], ot[:])

        # group-granular interleave; rescale + outproj are deferred one
        # iteration so their DVE work is dependency-free when the queue
        # reaches it (emitted at the tail of a chain they head-of-line-block
        # DVE on the last PV, stalling PE on PSUM evacuations)
        # Rescale placement: b0's runs immediately after its chain (DVE has
        # nothing pending during b1's chain, so the PV-end wait is harmless);
        # b1's is deferred into the next iteration, where its inputs are
        # long-ready — so it neither blocks the DVE queue nor outlives its
        # otp buffers (the 2-deep otp pool is only recycled by the next
        # iteration's chains, after the deferred rescale has run).
        r_b1 = None
        load_group(0, 0)
        load_group(1, 0)
        for n in range(QG):
            project_group(0, n)
            if r_b1 is not None:
                r_b1()
            project_group(1, n)
            if n > 0:
                for b in range(B):
                    outproj_qg(b, n - 1)
            # prefetch next iteration's inputs after the (short, quickly
            # cleared) out-DMAs, streaming in during this iteration's
            # attention chains
            if n + 1 < QG:
                load_group(0, n + 1)
                load_group(1, n + 1)
            r_b0 = attention_qg(0, n)
            r_b0()
            r_b1 = attention_qg(1, n)
        r_b1()
        for b in range(B):
            outproj_qg(b, QG - 1, tail=True)

    nc.compile()
    return nc


def kernel(q, k, v, Wq, Wk, Wv, Wo):
    global LAST_RESULTS
    import ml_dtypes
    from concourse.bass_utils import run_bass_kernel_spmd

    bf16 = ml_dtypes.bfloat16

    def chunk_T(x):
        # [N, C] -> xT [C, N] -> [128, KCH, N] with x[p, kk, t] = xT[kk*128+p, t]
        xT = np.asarray(x, np.float32).reshape(N, C).T
        return np.ascontiguousarray(
            xT.reshape(KCH, 128, N).transpose(1, 0, 2)).astype(bf16)

    qc, kc, vc = chunk_T(q), chunk_T(k), chunk_T(v)
    Wq = np.asarray(Wq, np.float32)
    Wk = np.asarray(Wk, np.float32)
    Wv = np.asarray(Wv, np.float32)
    Wo = np.asarray(Wo, np.float32)

    def chunk_W(W, sl):
        # Wc = W[sl, :].T [C, LD] -> [128, KCH, LD]
        WT = W[sl, :].T
        return np.ascontiguousarray(
            WT.reshape(KCH, 128, LD).transpose(1, 0, 2)).astype(bf16)

    in_maps = []
    for c in range(NCORES):
        sl = slice(c * LD, (c + 1) * LD)
        in_maps.append({
            "qT": qc, "kT": kc, "vT": vc,
            "wqT": chunk_W(Wq, sl),
            "wkT": chunk_W(Wk, sl),
            "wvT": chunk_W(Wv, sl),
            "woT": np.ascontiguousarray(Wo[:, sl].T).astype(bf16),
        })

    nc = _build_program()
    res = run_bass_kernel_spmd(nc, in_maps, list(range(NCORES)))
    LAST_RESULTS = res
    acc = np.zeros((N, C), np.float32)
    for rmap in res.results:
        acc += np.asarray(rmap["out"], np.float32)
    return acc.reshape(B, T, C)
